# revision 6
# baseline (speedup 1.0000x reference)
"""Binary-weight dense layer on 8 trn2 NeuronCores.

Computes out[b,s,f] = scale * sum_i x[b,s,i] * (kernel[i,f] ? +1 : -1)
for x [4, 4096, 1024] f32, kernel [1024, 1024] bool, scale scalar f32.

Strategy: data-parallel over the 16384 rows (2048 rows/core), pure bf16
matmul with scale folded into the +-1 weights (exact in bf16 for
power-of-two scales).  Host-side prep packs per-core tensors so every
DMA runs with >=2KB contiguous lines per partition (sub-1KB lines halve
HWDGE queue throughput):
  w    [128p, 8k, 1024n]  - 2KB lines per k-subtile instruction
  xg0  [128p, 8k,  512m]  - rows 0-511, 2KB lines per k-pair
  xr   [128p, 8k, 1536m]  - rows 512-2047, 1-2KB lines

Schedule (from NTFF trace analysis of the baseline):
- sync ring carries w then all output stores; scalar ring carries x.
  First-needed chunks are first in each queue; both queues run ~3
  instructions in flight with ~0.6us completion-sem latency.
- 12 bridge matmuls on a memset buffer keep the PE continuously busy
  (warming the HAM clock gate) until the first real operands land
  (~10us); any PE idle gap resets the warmup and the stream then runs
  at 1.2 GHz for another ~3.4us.
- Phase 1 k-major over m-tiles 0-3 consumes chunks in arrival order;
  phase 2 m-major for m-tiles 4-15.
- m-tiles 3 and 15 accumulate into two separate [128,512] PSUM tiles
  so the final tile's half-a eviction can overlap half-b matmuls
  without a false whole-tile WAR dependency (PSUM budget: 3x1024 +
  2x512 f32 = exactly 8 banks).
- PSUM f32 is converted to bf16 by the DVE eviction copy, halving
  output DMA bytes; the host upcasts to f32.
"""

import numpy as np
import ml_dtypes

import concourse.bacc as bacc
import concourse.mybir as mybir
import concourse.tile as tile
from concourse.bass_utils import run_bass_kernel_spmd

N_CORES = 8
B, S, K, N = 4, 4096, 1024, 1024
ROWS = B * S                     # 16384
ROWS_PER_CORE = ROWS // N_CORES  # 2048
P = 128                          # partitions
KT = K // P                      # 8 contraction subtiles
MT = ROWS_PER_CORE // P          # 16 row tiles per core
NHALF = 512                      # one PSUM bank of f32
G0 = 4                           # phase-1 m-tiles (PSUM holds exactly 4)
GROWS = G0 * P                   # 512 rows covered by phase 1
RROWS = ROWS_PER_CORE - GROWS    # 1536 rows covered by phase 2

_module_cache = {}


def build_module():
    nc = bacc.Bacc(None)
    xg0 = nc.dram_tensor("xg0", [P, KT, GROWS], mybir.dt.bfloat16,
                         kind="ExternalInput")
    xr = nc.dram_tensor("xr", [P, KT, RROWS], mybir.dt.bfloat16,
                        kind="ExternalInput")
    w = nc.dram_tensor("w", [P, KT, N], mybir.dt.bfloat16,
                       kind="ExternalInput")
    out = nc.dram_tensor("out", [ROWS_PER_CORE, N], mybir.dt.bfloat16,
                         kind="ExternalOutput")

    with tile.TileContext(nc) as tc:
        with (
            tc.tile_pool(name="persist", bufs=1) as persist,
            tc.tile_pool(name="psum", bufs=1, space="PSUM") as ps_pool,
            tc.tile_pool(name="outp", bufs=3) as out_pool,
        ):
            wu = persist.tile([P, 384], mybir.dt.bfloat16, tag="wu")
            nc.gpsimd.memset(wu, 0)

            XG = persist.tile([P, KT, GROWS], mybir.dt.bfloat16, tag="xg0",
                              name="xg0")
            XR = persist.tile([P, KT, RROWS], mybir.dt.bfloat16, tag="xr",
                              name="xr")
            W = persist.tile([P, KT, N], mybir.dt.bfloat16, tag="w", name="w")

            # --- DMA schedule.  Per-ring FIFO order == need order. ---
            # sync ring: w per k-subtile (2KB lines; k=0 gates the first
            # real matmul), then (emitted by evict()) the output stores.
            for k in range(KT):
                nc.sync.dma_start(out=W[:, k:k + 1, :], in_=w[:, k:k + 1, :])
            # scalar ring: phase-1 x in k-pairs (2KB lines), then phase-2
            # rows 512-1023 per k-pair, then rows 1024-2047 per k-pair.
            for k in range(0, KT, 2):
                nc.scalar.dma_start(out=XG[:, k:k + 2, :],
                                    in_=xg0[:, k:k + 2, :])
            for k in range(0, KT, 2):
                nc.scalar.dma_start(out=XR[:, k:k + 2, 0:GROWS],
                                    in_=xr[:, k:k + 2, 0:GROWS])
            for k in range(0, KT, 2):
                nc.scalar.dma_start(out=XR[:, k:k + 2, GROWS:RROWS],
                                    in_=xr[:, k:k + 2, GROWS:RROWS])

            # --- PSUM: m-tiles 0-2 (and phase-2 m%3 reuse) get full
            # [128,1024] tiles; m-tiles 3 and 15 use two [128,512] tiles.
            ps_full = {}
            for m in range(3):
                ps_full[m] = ps_pool.tile([P, N], mybir.dt.float32,
                                          tag=f"ps{m}", name=f"ps{m}")
            ps_half = [ps_pool.tile([P, NHALF], mybir.dt.float32,
                                    tag=f"psh{h}", name=f"psh{h}")
                       for h in range(2)]

            # Bridge matmuls (cold ~213ns each): PE continuously busy from
            # block entry (~7.2us) until real operands land (~10.2us).
            for _ in range(12):
                nc.tensor.matmul(ps_full[0][:, 0:256], wu[:, 0:P],
                                 wu[:, P:384], start=True, stop=True)

            def lhs(m, k):
                if m < G0:
                    return XG[:, k, m * P:(m + 1) * P]
                o = (m - G0) * P
                return XR[:, k, o:o + P]

            def mm(m, k, ps):
                lhsT = lhs(m, k)
                nc.tensor.matmul(ps[:, 0:NHALF], lhsT, W[:, k, 0:NHALF],
                                 start=(k == 0), stop=(k == KT - 1))
                nc.tensor.matmul(ps[:, NHALF:N], lhsT, W[:, k, NHALF:N],
                                 start=(k == 0), stop=(k == KT - 1))

            def mm_half(m, k, h, ps):
                nc.tensor.matmul(ps[:, 0:NHALF], lhs(m, k),
                                 W[:, k, h * NHALF:(h + 1) * NHALF],
                                 start=(k == 0), stop=(k == KT - 1))

            def evict(m, ps):
                ot = out_pool.tile([P, N], mybir.dt.bfloat16, tag="ot")
                nc.vector.tensor_copy(ot, ps)
                nc.sync.dma_start(out=out[m * P:(m + 1) * P, :], in_=ot)

            def evict_halves(m, ring_split):
                # copy/store each [128,512] PSUM tile separately so the
                # half-a store overlaps half-b work (no shared-tile WAR)
                ot = out_pool.tile([P, N], mybir.dt.bfloat16, tag="ot")
                for h in range(2):
                    lo, hi = h * NHALF, (h + 1) * NHALF
                    nc.vector.tensor_copy(ot[:, lo:hi], ps_half[h])
                    ring = (nc.sync if h == 0 else nc.scalar) if ring_split \
                        else nc.sync
                    ring.dma_start(out=out[m * P:(m + 1) * P, lo:hi],
                                   in_=ot[:, lo:hi])

            # Phase 1: m-tiles 0-3 k-major, consuming chunks as they
            # arrive.  m3 accumulates into the two half tiles.
            for k in range(KT):
                for m in range(3):
                    mm(m, k, ps_full[m])
                for h in range(2):
                    mm_half(3, k, h, ps_half[h])
            for m in range(3):
                evict(m, ps_full[m])
            evict_halves(3, ring_split=False)

            # Phase 2: m-tiles 4-14 m-major on the three full tiles;
            # m-tile 15 runs its halves back to back on the half tiles so
            # only the very last half's eviction trails the PE stream.
            for m in range(G0, MT - 1):
                ps = ps_pool.tile([P, N], mybir.dt.float32,
                                  tag=f"ps{(m - G0) % 3}", name=f"ps{m}")
                for k in range(KT):
                    mm(m, k, ps)
                evict(m, ps)
            m = MT - 1
            ps_half[0] = ps_pool.tile([P, NHALF], mybir.dt.float32,
                                      tag="psh0", name="psh0b")
            ps_half[1] = ps_pool.tile([P, NHALF], mybir.dt.float32,
                                      tag="psh1", name="psh1b")
            ot = out_pool.tile([P, N], mybir.dt.bfloat16, tag="ot")
            for h in range(2):
                lo, hi = h * NHALF, (h + 1) * NHALF
                for k in range(KT):
                    mm_half(m, k, h, ps_half[h])
                nc.vector.tensor_copy(ot[:, lo:hi], ps_half[h])
                ring = nc.sync if h == 0 else nc.scalar
                ring.dma_start(out=out[m * P:(m + 1) * P, lo:hi],
                               in_=ot[:, lo:hi])
    nc.finalize()
    return nc


def get_module():
    if "nc" not in _module_cache:
        _module_cache["nc"] = build_module()
    return _module_cache["nc"]


def _prepare_in_maps(x, kernel, scale):
    bf16 = ml_dtypes.bfloat16
    x2d = np.asarray(x, dtype=np.float32).reshape(ROWS, K)
    scale = np.float32(scale)
    w_signed = np.where(np.asarray(kernel, dtype=bool), scale, -scale)
    # w[p, k, n] = w_signed[k*128 + p, n]
    w_packed = np.ascontiguousarray(
        w_signed.reshape(KT, P, N).transpose(1, 0, 2).astype(bf16))
    in_maps = []
    for c in range(N_CORES):
        shard = x2d[c * ROWS_PER_CORE:(c + 1) * ROWS_PER_CORE]
        # xt[p, k, m] = shard[m, k*128 + p]
        xt_c = shard.T.reshape(KT, P, ROWS_PER_CORE).transpose(1, 0, 2)
        xg0_c = np.ascontiguousarray(xt_c[:, :, 0:GROWS].astype(bf16))
        xr_c = np.ascontiguousarray(xt_c[:, :, GROWS:].astype(bf16))
        in_maps.append({"xg0": xg0_c, "xr": xr_c, "w": w_packed})
    return in_maps


def kernel(x, kernel, scale):
    nc = get_module()
    in_maps = _prepare_in_maps(x, kernel, scale)
    res = run_bass_kernel_spmd(nc, in_maps, core_ids=list(range(N_CORES)))
    out = np.concatenate([r["out"] for r in res.results], axis=0)
    return out.astype(np.float32).reshape(B, S, N)
